# revision 1
# baseline (speedup 1.0000x reference)
"""LorentzBatchNorm2d Trainium2 kernel (8-core SPMD).

Input x: [64, 64, 64, 64] (B, C, H, W) float32, gamma/beta: [63].
Sharded data-parallel over B (8 planes per core). Channels live on SBUF
partitions (top pixel-half on partitions 0-63, bottom half on 64-127);
pixels along the free axis. Cross-channel reductions (Lorentz inner
product, sum of squares) run on the PE as matmuls that accumulate into a
single PSUM bank via per-tile diagonal-shifted weight matrices; the
per-pixel arccosh/coef chain runs batched over that bank. Two tiny
AllReduces: channel sums (64 floats) and the Frechet variance (1 float).
"""

import sys

sys.path.insert(0, "/opt/trn_rl_repo")

import numpy as np

import concourse.bass as bass
import concourse.tile as tile
from concourse import mybir

f32 = mybir.dt.float32
AF = mybir.ActivationFunctionType
ALU = mybir.AluOpType

B, C, H, W = 64, 64, 64, 64
EPS = 1e-5
NCORES = 8
PPC = B // NCORES          # planes (b indices) per core = 8
HWP = H * W                # pixels per plane = 4096
PIX = PPC * HWP            # pixels per core = 32768
HALF = PIX // 2            # 16384 per partition-half
F = 512                    # pixels per tile (one PSUM bank of fp32)
NT = HALF // F             # 32 tiles per core
N_TOTAL = B * H * W        # 262144

# dtype knobs for the PE streams (accuracy/speed tradeoff, measured on HW)
ALPHA_MM_DT = f32
RANK1_MM_DT = f32
TSUM_MM_DT = f32


def build_program(repeat: int = 1, phases: int = 3):
    nc = bass.Bass(num_devices=NCORES)

    x_d = nc.declare_dram_parameter("x", [PPC, C, HWP], f32, isOutput=False)
    out_d = nc.declare_dram_parameter("out", [PPC, C, HWP], f32, isOutput=True)

    # small constant inputs (built in numpy per call)
    sign_d = nc.declare_dram_parameter("sign_col", [128, 1], f32, isOutput=False)
    masktop_d = nc.declare_dram_parameter("masktop_col", [128, 1], f32, isOutput=False)
    maskbot_d = nc.declare_dram_parameter("maskbot_col", [128, 1], f32, isOutput=False)
    gneg_d = nc.declare_dram_parameter("gneg_col", [128, 1], f32, isOutput=False)
    beta_d = nc.declare_dram_parameter("beta_col", [128, 1], f32, isOutput=False)
    glhsT_d = nc.declare_dram_parameter("gamma_lhsT", [2, 128], f32, isOutput=False)
    maskal_d = nc.declare_dram_parameter("maskalpha_col", [128, 1], f32, isOutput=False)
    ident_d = nc.declare_dram_parameter("identity", [128, 128], f32, isOutput=False)
    # per-tile diagonal-shifted weight blocks:
    #   alhsT_base[:, 128t + 4t + {2,3}] = e0 patterns (x0 extractors);
    #   cols 128t+4t+{0,1} are filled with mu_signed at runtime.
    albase_d = nc.declare_dram_parameter("alhsT_base", [128, 128 * NT], f32, isOutput=False)
    #   tones[:, 128t + {t, 32+t}] = masktop/maskbot (per-tile sum rows)
    tones_d = nc.declare_dram_parameter("tones_all", [128, 128 * NT], f32, isOutput=False)

    # collective + scalar-broadcast bounce buffers (HBM)
    ar1_in = nc.dram_tensor("ar1_in", [64], f32)
    ar1_out = nc.dram_tensor("ar1_out", [64], f32, addr_space="Shared")
    ar2_in = nc.dram_tensor("ar2_in", [1], f32)
    ar2_out = nc.dram_tensor("ar2_out", [1], f32, addr_space="Shared")
    bnc = nc.dram_tensor("scalar_bounce", [4], f32)
    corr_bnc = nc.dram_tensor("corr_bounce", [2, 128], f32)

    rg = [list(range(NCORES))]

    from contextlib import ExitStack

    with tile.TileContext(nc) as tc:
        with ExitStack() as stack:
            resident = stack.enter_context(tc.tile_pool(name="resident", bufs=1))
            singles = stack.enter_context(tc.tile_pool(name="singles", bufs=1))
            work = stack.enter_context(tc.tile_pool(name="work", bufs=2))
            rep = stack.enter_context(tc.tile_pool(name="rep", bufs=2))
            psA = stack.enter_context(tc.tile_pool(name="psA", bufs=1, space="PSUM"))
            psT = stack.enter_context(tc.tile_pool(name="psT", bufs=1, space="PSUM"))
            psS = stack.enter_context(tc.tile_pool(name="psS", bufs=1, space="PSUM"))
            psP = stack.enter_context(tc.tile_pool(name="psP", bufs=1, space="PSUM"))
            psGC = stack.enter_context(tc.tile_pool(name="psGC", bufs=2, space="PSUM"))
            psCR = stack.enter_context(tc.tile_pool(name="psCR", bufs=2, space="PSUM"))

            x_sb = resident.tile([128, HALF], f32)
            out_s = resident.tile([128, HALF], f32)
            alhsT = resident.tile([128, 128 * NT], f32)
            tones = resident.tile([128, 128 * NT], f32)

            sign_c = singles.tile([128, 1], f32)
            masktop_c = singles.tile([128, 1], f32)
            maskbot_c = singles.tile([128, 1], f32)
            gneg_c = singles.tile([128, 1], f32)
            beta_c = singles.tile([128, 1], f32)
            glhsT_c = singles.tile([2, 128], f32)
            maskal_c = singles.tile([128, 1], f32)
            ident_c = singles.tile([128, 128], f32)
            for dst, src in (
                (sign_c, sign_d), (masktop_c, masktop_d), (maskbot_c, maskbot_d),
                (gneg_c, gneg_d), (beta_c, beta_d), (glhsT_c, glhsT_d),
                (maskal_c, maskal_d), (ident_c, ident_d),
                (alhsT, albase_d), (tones, tones_d),
            ):
                nc.sync.dma_start(out=dst[:], in_=src[:])

            zero_c = singles.tile([128, 1], f32)
            nc.vector.memset(zero_c[:], 0.0)
            neg1_c = singles.tile([128, 1], f32)
            nc.vector.memset(neg1_c[:], -1.0)
            pone_c = singles.tile([128, 1], f32)
            nc.vector.memset(pone_c[:], 1.0)

            # PE primers: absorb const-DMA waits on the PE clock one
            # semaphore at a time (instruction wait-slot limit workaround)
            prime_ps = psP.tile([1, 8 + repeat], f32)
            prime_n = [0]
            def pe_prime(col_ap):
                j = prime_n[0]
                prime_n[0] += 1
                nc.tensor.matmul(
                    out=prime_ps[0:1, j:j + 1], lhsT=col_ap, rhs=col_ap
                )
            pe_prime(ident_c[0:128, 0:1])
            pe_prime(alhsT[0:128, 0:1])
            pe_prime(tones[0:128, 0:1])
            pe_prime(glhsT_c[0:2, 0:1])

            # ---- load x (4 paired-plane DMAs: planes q and q+4 fill the
            # top/bottom partition halves of one 4096-pixel column block) ----
            for q in range(4):
                nc.sync.dma_start(
                    out=x_sb[:, q * HWP:(q + 1) * HWP], in_=x_d[q:PPC:4]
                )
            pe_prime(x_sb[0:128, 0:1])

            for _rep in range(repeat):
                # ---- P1: per-channel sums (ACT copy with free-axis accumulate;
                # out_s doubles as the throwaway sink) ----
                pcol = singles.tile([128, 4], f32)
                for q in range(4):
                    nc.scalar.activation(
                        out=out_s[:, q * HWP:(q + 1) * HWP],
                        in_=x_sb[:, q * HWP:(q + 1) * HWP], func=AF.Copy,
                        accum_out=pcol[:, q:q + 1],
                    )
                ssum = singles.tile([128, 1], f32)
                nc.vector.reduce_sum(out=ssum[:], in_=pcol[:], axis=mybir.AxisListType.X)
                # fold bottom half onto top: move partitions 64:128 -> 0:64, add
                tmp64 = singles.tile([64, 1], f32)
                nc.sync.dma_start(out=tmp64[:], in_=ssum[64:128, 0:1])
                s64 = singles.tile([64, 1], f32)
                nc.vector.tensor_add(out=s64[:], in0=ssum[0:64, 0:1], in1=tmp64[:])

                # ---- AR1: global channel sums ----
                nc.sync.dma_start(out=ar1_in[:], in_=s64[:])
                nc.gpsimd.collective_compute(
                    "AllReduce", ALU.add, replica_groups=rg,
                    ins=[ar1_in[:]], outs=[ar1_out[:]],
                )
                Sg = singles.tile([128, 1], f32)
                nc.sync.dma_start(
                    out=Sg[:],
                    in_=bass.AP(tensor=ar1_out, offset=0, ap=[[0, 2], [1, 64]]),
                )

                # ---- mu chain (tiny) ----
                # u = 2*S0^2 - sum(S^2)  (= N^2 * (m0^2 - |m_s|^2))
                ss_ps = psS.tile([1, 1], f32, tag="small")
                nc.tensor.matmul(out=ss_ps[:], lhsT=Sg[0:64, 0:1], rhs=Sg[0:64, 0:1])
                ss_sb = singles.tile([1, 1], f32)
                nc.scalar.copy(out=ss_sb[:], in_=ss_ps[:])
                q11 = singles.tile([1, 1], f32)
                nc.vector.tensor_mul(out=q11[:], in0=Sg[0:1, 0:1], in1=Sg[0:1, 0:1])
                u11 = singles.tile([1, 1], f32)
                nc.vector.tensor_scalar(
                    out=u11[:], in0=q11[:], scalar1=2.0, scalar2=ss_sb[:],
                    op0=ALU.mult, op1=ALU.subtract,
                )
                nc.vector.tensor_scalar_max(
                    out=u11[:], in0=u11[:], scalar1=EPS * float(N_TOTAL) ** 2
                )
                nc.scalar.activation(out=u11[:], in_=u11[:], func=AF.Sqrt, bias=zero_c[0:1])
                rs11 = singles.tile([1, 1], f32)
                nc.vector.reciprocal(out=rs11[:], in_=u11[:])   # rsqrt(mm)/N
                nc.sync.dma_start(out=bnc[0:1], in_=rs11[:])
                rs_col = singles.tile([128, 1], f32)
                nc.sync.dma_start(
                    out=rs_col[:],
                    in_=bass.AP(tensor=bnc, offset=0, ap=[[0, 128], [1, 1]]),
                )
                mu_col = singles.tile([128, 1], f32)
                nc.vector.tensor_mul(out=mu_col[:], in0=Sg[:], in1=rs_col[:])
                mus_col = singles.tile([128, 1], f32)
                nc.vector.tensor_mul(out=mus_col[:], in0=mu_col[:], in1=sign_c[:])
                # inv1m = 1/(1+mu0)
                i11 = singles.tile([1, 1], f32)
                nc.vector.tensor_scalar_add(out=i11[:], in0=mu_col[0:1, 0:1], scalar1=1.0)
                inv11 = singles.tile([1, 1], f32)
                nc.vector.reciprocal(out=inv11[:], in_=i11[:])
                nc.sync.dma_start(out=bnc[1:2], in_=inv11[:])
                inv_col = singles.tile([128, 1], f32)
                nc.sync.dma_start(
                    out=inv_col[:],
                    in_=bass.AP(tensor=bnc, offset=1, ap=[[0, 128], [1, 1]]),
                )
                # w0n = -gamma * mu_s * inv1m  (zero on channel-0 rows via gneg)
                w0a = singles.tile([128, 1], f32)
                nc.vector.tensor_mul(out=w0a[:], in0=gneg_c[:], in1=mu_col[:])
                w0n = singles.tile([128, 1], f32)
                nc.vector.tensor_mul(out=w0n[:], in0=w0a[:], in1=inv_col[:])
                # masked split + transpose into corr lhsT [2, 128]
                wt_c = singles.tile([128, 1], f32)
                nc.vector.tensor_mul(out=wt_c[:], in0=w0n[:], in1=masktop_c[:])
                wb_c = singles.tile([128, 1], f32)
                nc.vector.tensor_mul(out=wb_c[:], in0=w0n[:], in1=maskbot_c[:])
                wTa_ps = psS.tile([1, 128], f32, tag="small")
                nc.tensor.transpose(out=wTa_ps[:], in_=wt_c[:], identity=ident_c[:])
                wTa_sb = singles.tile([1, 128], f32)
                nc.scalar.copy(out=wTa_sb[:], in_=wTa_ps[:])
                wTb_ps = psS.tile([1, 128], f32, tag="small")
                nc.tensor.transpose(out=wTb_ps[:], in_=wb_c[:], identity=ident_c[:])
                wTb_sb = singles.tile([1, 128], f32)
                nc.scalar.copy(out=wTb_sb[:], in_=wTb_ps[:])
                corr_lhsT = singles.tile([2, 128], f32)
                nc.sync.dma_start(out=corr_bnc[0:1, :], in_=wTa_sb[:])
                nc.sync.dma_start(out=corr_bnc[1:2, :], in_=wTb_sb[:])
                nc.sync.dma_start(out=corr_lhsT[:], in_=corr_bnc[:])
                pe_prime(corr_lhsT[0:2, 0:1])
                # masked mu columns for the alpha weights
                mut_c = singles.tile([128, 1], f32)
                nc.vector.tensor_mul(out=mut_c[:], in0=mus_col[:], in1=masktop_c[:])
                mub_c = singles.tile([128, 1], f32)
                nc.vector.tensor_mul(out=mub_c[:], in0=mus_col[:], in1=maskbot_c[:])
                # scatter mu columns into the per-tile weight blocks:
                # col(t) = 132*t (+1 for the bottom-half column)
                al_ap = alhsT[:]
                for off, src_c in ((0, mut_c), (1, mub_c)):
                    dst = bass.AP(
                        tensor=al_ap.tensor, offset=al_ap.offset + off,
                        ap=[[128 * NT, 128], [132, NT], [1, 1]],
                    )
                    sap = src_c[:]
                    srcb = bass.AP(
                        tensor=sap.tensor, offset=sap.offset,
                        ap=[[1, 128], [0, NT], [1, 1]],
                    )
                    nc.vector.tensor_copy(out=dst, in_=srcb)

                if phases >= 2:
                    # ---- P2: alpha/x0 matmuls accumulated into one PSUM bank ----
                    # tile t writes rows (4t, 4t+1, 4t+2, 4t+3) = (a_top, a_bot,
                    # x0_top, x0_bot); all other rows of its product are zero.
                    apsum = psA.tile([128, F], f32)
                    for t in range(NT):
                        nc.tensor.matmul(
                            out=apsum[:],
                            lhsT=alhsT[:, 128 * t:128 * (t + 1)].bitcast(ALPHA_MM_DT),
                            rhs=x_sb[:, t * F:(t + 1) * F].bitcast(ALPHA_MM_DT),
                            start=(t == 0), stop=(t == NT - 1),
                            skip_group_check=True,
                        )
                    abank = singles.tile([128, F], f32)
                    nc.scalar.copy(out=abank[:], in_=apsum[:])

                    # ---- batched per-pixel chain ----
                    cb = singles.tile([128, F], f32)
                    nc.vector.tensor_scalar_max(out=cb[:], in0=abank[:], scalar1=1.0 + EPS)
                    shift_sb = singles.tile([126, F], f32)
                    nc.sync.dma_start(out=shift_sb[:], in_=abank[2:128, :])
                    s_sb = singles.tile([126, F], f32)
                    nc.vector.tensor_add(out=s_sb[:], in0=cb[0:126, :], in1=shift_sb[:])
                    q_sb = singles.tile([128, F], f32)
                    nc.scalar.activation(out=q_sb[:], in_=cb[:], func=AF.Square, bias=zero_c[:])
                    sq_sb = singles.tile([128, F], f32)
                    nc.scalar.activation(out=sq_sb[:], in_=q_sb[:], func=AF.Sqrt, bias=neg1_c[:])
                    t1_sb = singles.tile([128, F], f32)
                    nc.vector.tensor_add(out=t1_sb[:], in0=cb[:], in1=sq_sb[:])
                    d_sb = singles.tile([128, F], f32)
                    nc.scalar.activation(out=d_sb[:], in_=t1_sb[:], func=AF.Ln, bias=zero_c[:])
                    r_sb = singles.tile([128, F], f32)
                    nc.vector.reciprocal(out=r_sb[:], in_=sq_sb[:])
                    cf_sb = singles.tile([128, F], f32, tag="t1_sb")
                    nc.vector.tensor_mul(out=cf_sb[:], in0=d_sb[:], in1=r_sb[:])
                    cr_sb = singles.tile([128, F], f32, tag="shift_sb")
                    nc.vector.tensor_mul(out=cr_sb[0:126, :], in0=cf_sb[0:126, :], in1=s_sb[:])
                    # d^2 sums (mask junk rows, then reduce over partitions via PE)
                    dcol = singles.tile([128, 1], f32)
                    nc.scalar.activation(
                        out=q_sb[:], in_=d_sb[:], func=AF.Square, bias=zero_c[:],
                        accum_out=dcol[:],
                    )
                    dmask = singles.tile([128, 1], f32)
                    nc.vector.tensor_mul(out=dmask[:], in0=dcol[:], in1=maskal_c[:])
                    dsq_ps = psS.tile([1, 1], f32, tag="small")
                    nc.tensor.matmul(out=dsq_ps[:], lhsT=dmask[:], rhs=pone_c[:])
                    dsq_sb = singles.tile([1, 1], f32)
                    nc.scalar.copy(out=dsq_sb[:], in_=dsq_ps[:])

                    # ---- AR2: Frechet variance ----
                    nc.sync.dma_start(out=ar2_in[:], in_=dsq_sb[:])
                    nc.gpsimd.collective_compute(
                        "AllReduce", ALU.add, replica_groups=rg,
                        ins=[ar2_in[:]], outs=[ar2_out[:]],
                    )
                    vg = singles.tile([1, 1], f32)
                    nc.sync.dma_start(out=vg[:], in_=ar2_out[:])
                    nc.scalar.activation(
                        out=vg[:], in_=vg[:], func=AF.Sqrt, bias=zero_c[0:1],
                        scale=1.0 / float(N_TOTAL),
                    )
                    nc.vector.tensor_scalar_add(out=vg[:], in0=vg[:], scalar1=EPS)
                    iv11 = singles.tile([1, 1], f32)
                    nc.vector.reciprocal(out=iv11[:], in_=vg[:])
                    nc.sync.dma_start(out=bnc[2:3], in_=iv11[:])
                    invsd_col = singles.tile([128, 1], f32)
                    nc.sync.dma_start(
                        out=invsd_col[:],
                        in_=bass.AP(tensor=bnc, offset=2, ap=[[0, 128], [1, 1]]),
                    )
                    invsd2 = singles.tile([128, 1], f32)
                    nc.vector.tensor_copy(out=invsd2[:], in_=invsd_col[:])
                    beta2 = singles.tile([128, 1], f32)
                    nc.vector.tensor_copy(out=beta2[:], in_=beta_c[:])

                if phases >= 3:
                    # ---- P3: per-tile output assembly ----
                    tpsum = psT.tile([128, F], f32)
                    for t in range(NT):
                        xs = x_sb[:, t * F:(t + 1) * F]
                        cf2c = rep.tile([2, F], f32, tag="cf2c")
                        cr2c = rep.tile([2, F], f32, tag="cr2c")
                        nc.sync.dma_start(out=cf2c[:], in_=cf_sb[4 * t:4 * t + 2, :])
                        nc.sync.dma_start(out=cr2c[:], in_=cr_sb[4 * t:4 * t + 2, :])
                        gc_ps = psGC.tile([128, F], f32, tag="gc")
                        nc.tensor.matmul(
                            out=gc_ps[:],
                            lhsT=glhsT_c[:].bitcast(RANK1_MM_DT),
                            rhs=cf2c[:].bitcast(RANK1_MM_DT),
                        )
                        e2 = work.tile([128, F], f32, tag="e2")
                        nc.vector.tensor_mul(out=e2[:], in0=xs, in1=gc_ps[:])
                        cr_ps = psCR.tile([128, F], f32, tag="cr")
                        nc.tensor.matmul(
                            out=cr_ps[:],
                            lhsT=corr_lhsT[:].bitcast(RANK1_MM_DT),
                            rhs=cr2c[:].bitcast(RANK1_MM_DT),
                        )
                        pre = work.tile([128, F], f32, tag="pre")
                        nc.vector.tensor_add(out=pre[:], in0=e2[:], in1=cr_ps[:])
                        so = out_s[:, t * F:(t + 1) * F]
                        nc.scalar.activation(
                            out=so, in_=pre[:], func=AF.Identity,
                            bias=beta2[:], scale=invsd2[:],
                        )
                        sq2 = work.tile([128, F], f32, tag="sq2")
                        nc.gpsimd.tensor_mul(out=sq2[:], in0=so, in1=so)
                        # t-sums: tile t's weight block puts sum_top at row t and
                        # sum_bot at row 32+t of the accumulating bank
                        nc.tensor.matmul(
                            out=tpsum[:],
                            lhsT=tones[:, 128 * t:128 * (t + 1)].bitcast(TSUM_MM_DT),
                            rhs=sq2[:].bitcast(TSUM_MM_DT),
                            start=(t == 0), stop=(t == NT - 1),
                            skip_group_check=True,
                        )

                    t_sb = singles.tile([64, F], f32)
                    nc.scalar.activation(
                        out=t_sb[:], in_=tpsum[0:64, :], func=AF.Sqrt, bias=pone_c[0:64]
                    )

                    # ---- output DMAs ----
                    for q in range(4):
                        sl = slice(q * HWP, (q + 1) * HWP)
                        nc.sync.dma_start(out=out_d[q, 1:64, :], in_=out_s[1:64, sl])
                        nc.sync.dma_start(out=out_d[4 + q, 1:64, :], in_=out_s[65:128, sl])
                        nc.sync.dma_start(
                            out=out_d[q, 0, :], in_=t_sb[8 * q:8 * q + 8, :]
                        )
                        nc.sync.dma_start(
                            out=out_d[4 + q, 0, :], in_=t_sb[32 + 8 * q:40 + 8 * q, :]
                        )

    return nc


def make_const_inputs(gamma: np.ndarray, beta: np.ndarray) -> dict:
    sign = np.ones((128, 1), np.float32)
    sign[1:64] = -1.0
    sign[65:128] = -1.0
    masktop = np.zeros((128, 1), np.float32)
    masktop[0:64] = 1.0
    maskbot = np.zeros((128, 1), np.float32)
    maskbot[64:128] = 1.0
    gneg = np.zeros((128, 1), np.float32)
    gneg[1:64, 0] = -gamma
    gneg[65:128, 0] = -gamma
    beta_col = np.zeros((128, 1), np.float32)
    beta_col[1:64, 0] = beta
    beta_col[65:128, 0] = beta
    glhsT = np.zeros((2, 128), np.float32)
    glhsT[0, 1:64] = gamma
    glhsT[1, 65:128] = gamma
    maskal = np.zeros((128, 1), np.float32)
    maskal[0::4] = 1.0
    maskal[1::4] = 1.0
    ident = np.eye(128, dtype=np.float32)
    albase = np.zeros((128, 128 * NT), np.float32)
    tones = np.zeros((128, 128 * NT), np.float32)
    for t in range(NT):
        albase[0, 128 * t + 4 * t + 2] = 1.0     # x0_top extractor
        albase[64, 128 * t + 4 * t + 3] = 1.0    # x0_bot extractor
        tones[0:64, 128 * t + t] = 1.0           # sum_top -> row t
        tones[64:128, 128 * t + 32 + t] = 1.0    # sum_bot -> row 32+t
    return {
        "sign_col": sign, "masktop_col": masktop, "maskbot_col": maskbot,
        "gneg_col": gneg, "beta_col": beta_col, "gamma_lhsT": glhsT,
        "maskalpha_col": maskal, "identity": ident,
        "alhsT_base": albase, "tones_all": tones,
    }




def _legalize_waits(nc):
    """Split multi-wait sync_info into standalone single-wait
    EventSemaphore instructions: the walrus codegen in this toolchain
    only encodes one sync-wait command per engine instruction."""
    n = 0
    for fn in nc.m.functions:
        for bb in fn.blocks:
            insts = bb.instructions
            i = 0
            while i < len(insts):
                ins = insts[i]
                si = getattr(ins, "sync_info", None)
                if si is not None and si.on_wait and len(si.on_wait) > 1:
                    waits = list(si.on_wait)
                    for w in waits[:-1]:
                        ev = mybir.InstEventSemaphore(
                            name=f"WSPLIT-{n}", engine=ins.engine,
                            ins=[], outs=[],
                            sync_info=mybir.SyncInfo(on_wait=[w], on_update=[]),
                        )
                        n += 1
                        insts.insert(i, ev)
                        i += 1
                    ins.sync_info = mybir.SyncInfo(
                        on_wait=[waits[-1]], on_update=list(si.on_update or [])
                    )
                i += 1
    return n


_PROGRAM = None


def _get_program():
    global _PROGRAM
    if _PROGRAM is None:
        _PROGRAM = build_program()
        _legalize_waits(_PROGRAM)
    return _PROGRAM




_RUNNER = None


def _get_runner():
    """Cached jitted SPMD executor (mirrors bass2jax.run_bass_via_pjrt's
    axon path, but reuses one jax.jit executable across calls)."""
    global _RUNNER
    if _RUNNER is not None:
        return _RUNNER
    import jax
    import jax.numpy as jnp  # noqa: F401
    from jax.experimental.shard_map import shard_map
    from jax.sharding import Mesh, PartitionSpec
    from concourse import bass2jax, mybir as _mb

    nc = _get_program()
    bass2jax.install_neuronx_cc_hook()
    partition_name = (
        nc.partition_id_tensor.name if nc.partition_id_tensor else None
    )
    in_names, out_names, out_avals, zero_outs = [], [], [], []
    for alloc in nc.m.functions[0].allocations:
        if not isinstance(alloc, _mb.MemoryLocationSet):
            continue
        name = alloc.memorylocations[0].name
        if alloc.kind == "ExternalInput":
            if name != partition_name:
                in_names.append(name)
        elif alloc.kind == "ExternalOutput":
            shape = tuple(alloc.tensor_shape)
            dtype = _mb.dt.np(alloc.dtype)
            out_names.append(name)
            out_avals.append(jax.core.ShapedArray(shape, dtype))
            zero_outs.append(np.zeros(shape, dtype))
    n_params = len(in_names)
    n_outs = len(out_avals)
    all_in_names = list(in_names) + list(out_names)
    if partition_name is not None:
        all_in_names.append(partition_name)
    donate = tuple(range(n_params, n_params + n_outs))

    def _body(*args):
        operands = list(args)
        if partition_name is not None:
            operands.append(bass2jax.partition_id_tensor())
        outs = bass2jax._bass_exec_p.bind(
            *operands,
            out_avals=tuple(out_avals),
            in_names=tuple(all_in_names),
            out_names=tuple(out_names),
            lowering_input_output_aliases=(),
            sim_require_finite=True,
            sim_require_nnan=True,
            nc=nc,
        )
        return tuple(outs)

    devices = jax.devices()[:NCORES]
    mesh = Mesh(np.asarray(devices), ("core",))
    in_specs = (PartitionSpec("core"),) * (n_params + n_outs)
    out_specs = (PartitionSpec("core"),) * n_outs
    sharded = jax.jit(
        shard_map(
            _body, mesh=mesh, in_specs=in_specs, out_specs=out_specs,
            check_rep=False,
        ),
        donate_argnums=donate,
        keep_unused=True,
    )

    def run(in_maps):
        per_core = [[np.asarray(m[n]) for n in in_names] for m in in_maps]
        concat_in = [
            np.concatenate([per_core[c][i] for c in range(NCORES)], axis=0)
            for i in range(n_params)
        ]
        concat_zeros = [
            np.zeros((NCORES * z.shape[0], *z.shape[1:]), z.dtype)
            for z in zero_outs
        ]
        out_arrs = sharded(*concat_in, *concat_zeros)
        return [
            {
                name: np.asarray(out_arrs[i]).reshape(
                    NCORES, *out_avals[i].shape
                )[c]
                for i, name in enumerate(out_names)
            }
            for c in range(NCORES)
        ]

    _RUNNER = (run, sharded, in_names, out_names, out_avals, zero_outs)
    return _RUNNER


def kernel(x: np.ndarray, gamma: np.ndarray, beta: np.ndarray) -> np.ndarray:
    run = _get_runner()[0]
    consts = make_const_inputs(
        np.asarray(gamma, np.float32), np.asarray(beta, np.float32)
    )
    x = np.asarray(x, np.float32)
    in_maps = []
    for k in range(NCORES):
        shard = np.ascontiguousarray(
            x[k * PPC:(k + 1) * PPC].reshape(PPC, C, HWP)
        )
        in_maps.append({"x": shard, **consts})
    results = run(in_maps)
    out = np.empty((B, C, H, W), np.float32)
    for k in range(NCORES):
        out[k * PPC:(k + 1) * PPC] = results[k]["out"].reshape(PPC, C, H, W)
    return out


if __name__ == "__main__":
    rng = np.random.default_rng(0)
    xs = rng.standard_normal((B, C - 1, H, W), np.float32) * 0.5
    x0 = np.sqrt(1.0 + np.sum(xs * xs, axis=1, keepdims=True))
    x = np.concatenate([x0, xs], axis=1).astype(np.float32)
    gamma = 0.5 + rng.random(C - 1, dtype=np.float32)
    beta = 0.05 * rng.standard_normal(C - 1).astype(np.float32)
    out = kernel(x=x, gamma=gamma, beta=beta)
    print(out.shape, out.dtype, np.isfinite(out).all())

